# revision 1
# baseline (speedup 1.0000x reference)
"""Multi-region RNN kernel for Trainium2 (8 NeuronCores, SPMD batch-sharded).

Model (per step t):
    inp  = einsum('bi,rih->rbh', x_t, W_ih)
    loc  = einsum('rbh,rhg->rbg', H, W_hh)
    msg  = einsum('ij,ibh->jbh', C, H)
    cross= einsum('rbh,rhg->rbg', msg, W_rhh)
    H'   = tanh(inp + loc + cross + bias)
Output: stack H over t -> [T,B,R*H] @ W_out + b_out.

Distribution: pure data-parallel over batch (B=32 -> 4 per core), parameters
replicated; no cross-core communication. Per core:
  Phase 1: input drive for all t precomputed as per-region matmuls
           (W_ih[r] stationary, x^T moving), bias folded in, staged to DRAM
           in [t][h,(b,r)] layout.
  Phase 2: sequential recurrence. State kept as bf16 [h=128, (b,r)=400] tiles.
           Per step: 100 loc MMs + 4 DMA-transposes (state -> region-major) +
           4 msg MMs (lhsT=H region-major, rhs=C) + 100 cross MMs, all
           accumulating in one PSUM bank slice-per-region; then DVE add of the
           staged input drive and ScalarE tanh. bf16 operands, fp32 PSUM.
  Phase 3: output projection from the bf16 state history (DRAM) with
           per-region accumulation into PSUM over (t,b)-blocks of 128;
           b_out added via a K=1 matmul of ones x b_out.
"""

import numpy as np
import ml_dtypes
from contextlib import ExitStack

import concourse.bass as bass
import concourse.bacc as bacc
import concourse.tile as tile
from concourse import mybir
from concourse.bass_utils import run_bass_kernel_spmd

T, B, I, H, R, O = 128, 32, 128, 128, 100, 64
NCORES = 8
BL = B // NCORES          # batch per core = 4
BR = BL * R               # state free size = 400, col = b*R + r
TB = T * BL               # 512
TBLK = 32                 # t-steps per phase-1/3 block -> 128 (t,b) cols
RPAD = 128                # region stride in state layout (DMA transpose needs x128)
BRP = BL * RPAD           # padded state free size = 512, col = b*RPAD + r

BF = mybir.dt.bfloat16
F32 = mybir.dt.float32
Act = mybir.ActivationFunctionType

# Enable walrus LDWEIGHTS optimization (off by default in compile_bir_kernel);
# the recurrence is weight-load bound, so LDW pipelining is the main lever.
import os as _os
if _os.environ.get("KERNEL_LDW_OPT", "0") == "1":
    import concourse.bass_utils as _bu
    if not getattr(_bu, "_ldw_opt_patched", False):
        _orig_run_command = _bu.run_command

        def _run_command_ldw(argv, **kwargs):
            argv = ["--enable-ldw-opt=true" if a == "--enable-ldw-opt=false" else a
                    for a in argv]
            return _orig_run_command(argv, **kwargs)

        _bu.run_command = _run_command_ldw
        _bu._ldw_opt_patched = True

_CACHE: dict = {}
NREP = 1   # test-only hook: repeat the whole body to measure device time deltas


def _build_program():
    nc = bacc.Bacc(None, target_bir_lowering=False)

    xT_d = nc.dram_tensor("xT", [I, TB], BF, kind="ExternalInput")        # [i,(t,b)]
    C_d = nc.dram_tensor("C", [R, R], BF, kind="ExternalInput")           # [i,j]
    Whh_d = nc.dram_tensor("Whh", [H, R * H], BF, kind="ExternalInput")   # [h,(r,g)]
    Wrhh_d = nc.dram_tensor("Wrhh", [H, R * H], BF, kind="ExternalInput")
    Wih_d = nc.dram_tensor("Wih", [I, R * H], BF, kind="ExternalInput")
    Wout_d = nc.dram_tensor("Wout", [H, R * O], BF, kind="ExternalInput")  # [h,(r,o)]
    biasT_d = nc.dram_tensor("biasT", [H, R], F32, kind="ExternalInput")
    bout_d = nc.dram_tensor("bout", [1, O], BF, kind="ExternalInput")
    out_d = nc.dram_tensor("out", [T, BL, O], F32, kind="ExternalOutput")

    with tile.TileContext(nc) as tc, ExitStack() as ctx:
        consts = ctx.enter_context(tc.tile_pool(name="consts", bufs=1))
        dram = ctx.enter_context(tc.tile_pool(name="dram", bufs=1, space="DRAM"))

        Whh_s = consts.tile([H, R * H], BF)
        nc.sync.dma_start(Whh_s[:], Whh_d[:])
        Wrhh_s = consts.tile([H, R * H], BF)
        nc.sync.dma_start(Wrhh_s[:], Wrhh_d[:])
        Wih_s = consts.tile([I, R * H], BF)
        nc.sync.dma_start(Wih_s[:], Wih_d[:])
        Wout_s = consts.tile([H, R * O], BF)
        nc.sync.dma_start(Wout_s[:], Wout_d[:])
        xT_s = consts.tile([I, TB], BF)
        nc.sync.dma_start(xT_s[:], xT_d[:])
        C_s = consts.tile([R, R], BF)
        nc.sync.dma_start(C_s[:], C_d[:])
        biasT_s = consts.tile([H, R], F32)
        nc.sync.dma_start(biasT_s[:], biasT_d[:])
        bout_s = consts.tile([1, O], BF)
        nc.sync.dma_start(bout_s[:], bout_d[:])
        ones_s = consts.tile([1, H], BF)
        nc.vector.memset(ones_s[:], 1.0)

        def _emit_body(_rep, bctx):
            inp_dram = dram.tile([T, H, BRP], F32, name=f"inp_dram{_rep}")
            hist_dram = dram.tile([T, H, BRP], BF, name=f"hist_dram{_rep}")
            inp4d = inp_dram.rearrange("t h (b r) -> t h b r", r=RPAD)
            hist4d = hist_dram.rearrange("t h (b r) -> t h b r", r=RPAD)

            # ---------------- Phase 1: input drive ----------------
            NBLK = T // TBLK
            p1_ps = bctx.enter_context(tc.tile_pool(name=f"p1_ps{_rep}", bufs=2, space="PSUM"))
            p1_st = bctx.enter_context(tc.tile_pool(name=f"p1_st{_rep}", bufs=1))
            for tb in range(NBLK):
                stage = p1_st.tile([H, TBLK * BR], F32, tag="p1stage")
                stage4 = stage.rearrange("h (t b r) -> h t b r", b=BL, r=R)
                for r in range(R):
                    ps = p1_ps.tile([H, TBLK * BL], F32, tag="p1psum")
                    nc.tensor.matmul(
                        ps[:],
                        Wih_s[:, r * H:(r + 1) * H],
                        xT_s[:, tb * TBLK * BL:(tb + 1) * TBLK * BL],
                        start=True, stop=True,
                    )
                    nc.scalar.activation(
                        out=stage4[:, :, :, r],
                        in_=ps.rearrange("h (t b) -> h t b", b=BL),
                        func=Act.Identity,
                        bias=biasT_s[:, r:r + 1],
                        scale=1.0,
                    )
                for b in range(BL):
                    nc.sync.dma_start(
                        out=inp4d[tb * TBLK:(tb + 1) * TBLK, :, b, 0:R].rearrange(
                            "t h r -> h t r"),
                        in_=stage4[:, :, b, :],
                    )

            # ---------------- Phase 2: recurrence ----------------
            st_pool = bctx.enter_context(tc.tile_pool(name=f"st{_rep}", bufs=3))
            hrm_pool = bctx.enter_context(tc.tile_pool(name=f"hrm{_rep}", bufs=2))
            msg_pool = bctx.enter_context(tc.tile_pool(name=f"msgp{_rep}", bufs=2))
            pre_pool = bctx.enter_context(tc.tile_pool(name=f"prep{_rep}", bufs=2))
            inp_pool = bctx.enter_context(tc.tile_pool(name=f"inpp{_rep}", bufs=3))
            ps_act = bctx.enter_context(tc.tile_pool(name=f"ps_act{_rep}", bufs=2, space="PSUM"))
            ps_msg = bctx.enter_context(tc.tile_pool(name=f"ps_msg{_rep}", bufs=2, space="PSUM"))

            Hprev = st_pool.tile([H, BRP], BF, tag="hstate")
            nc.vector.memset(Hprev[:], 0.0)

            for t in range(T):
                # bulk streaming on SWDGE queues keeps the two HWDGE queues
                # (SP, ACT) free for the latency-critical state transposes
                inp_t = inp_pool.tile([H, BRP], F32, tag="inp_t")
                nc.gpsimd.dma_start(inp_t[:], inp_dram[t, :, :])

                pa = ps_act.tile([H, BRP], F32, tag="pa")
                paR = pa.rearrange("h (b r) -> h b r", r=RPAD)
                HprevR = Hprev.rearrange("h (b r) -> h b r", r=RPAD)

                # state -> region-major [i, (b,h)] via per-batch DMA
                # transposes, spread across both HWDGE queues
                Hrm = hrm_pool.tile([RPAD, BL * H], BF, tag="hrm")
                for b, eng in zip(range(BL),
                                  (nc.sync, nc.scalar, nc.sync, nc.scalar)):
                    eng.dma_start(
                        out=Hrm[:, b * H:(b + 1) * H],
                        in_=Hprev[:, b * RPAD:(b + 1) * RPAD],
                        transpose=True,
                    )

                # local recurrence: per-region W_hh.
                # PSUM start=True clears the whole bank's has_written flags, so
                # only the first matmul into this tile may set it; later matmuls
                # write fresh columns / accumulate based on per-element flags.
                # First half of loc runs while the transposes land; the msg
                # matmuls slot in mid-stream so their ACT eviction overlaps the
                # second loc half, and cross starts without a PE stall.
                for r in range(R // 2):
                    nc.tensor.matmul(
                        paR[:, :, r],
                        Whh_s[:, r * H:(r + 1) * H],
                        HprevR[:, :, r],
                        start=(r == 0), stop=False,
                    )

                # message: msg_b^T[h,j] = sum_i H_b[i,h] C[i,j]
                pm = ps_msg.tile([H, BRP], F32, tag="pm")
                for b in range(BL):
                    nc.tensor.matmul(
                        pm[:, b * RPAD:b * RPAD + R],
                        Hrm[0:R, b * H:(b + 1) * H],
                        C_s[:],
                        start=(b == 0), stop=(b == BL - 1),
                    )
                Msg = msg_pool.tile([H, BRP], BF, tag="msg")
                nc.scalar.activation(out=Msg[:], in_=pm[:], func=Act.Copy, scale=1.0)
                MsgR = Msg.rearrange("h (b r) -> h b r", r=RPAD)

                for r in range(R // 2, R):
                    nc.tensor.matmul(
                        paR[:, :, r],
                        Whh_s[:, r * H:(r + 1) * H],
                        HprevR[:, :, r],
                        start=False, stop=False,
                    )

                # cross term: per-region W_rhh on the mixed state
                for r in range(R):
                    nc.tensor.matmul(
                        paR[:, :, r],
                        Wrhh_s[:, r * H:(r + 1) * H],
                        MsgR[:, :, r],
                        start=False, stop=(r == R - 1),
                    )

                # add input drive + tanh, split by region halves: the first
                # half of the next step's loc matmuls only needs the first
                # half of the state, so the PE restarts while half 2 activates
                Pre = pre_pool.tile([H, BRP], F32, tag="pre")
                Hnext = st_pool.tile([H, BRP], BF, tag="hstate")
                PreR = Pre.rearrange("h (b r) -> h b r", r=RPAD)
                HnextR = Hnext.rearrange("h (b r) -> h b r", r=RPAD)
                inpR = inp_t.rearrange("h (b r) -> h b r", r=RPAD)
                for lo, hi in ((0, R // 2), (R // 2, RPAD)):
                    nc.vector.tensor_tensor(
                        PreR[:, :, lo:hi], paR[:, :, lo:hi], inpR[:, :, lo:hi],
                        mybir.AluOpType.add)
                    nc.scalar.activation(out=HnextR[:, :, lo:hi],
                                         in_=PreR[:, :, lo:hi], func=Act.Tanh)
                nc.gpsimd.dma_start(out=hist_dram[t, :, :], in_=Hnext[:])
                Hprev = Hnext

            # ---------------- Phase 3: output projection ----------------
            p3_hh = bctx.enter_context(tc.tile_pool(name=f"p3_hh{_rep}", bufs=2))
            p3_ps = bctx.enter_context(tc.tile_pool(name=f"p3_ps{_rep}", bufs=2, space="PSUM"))
            p3_ot = bctx.enter_context(tc.tile_pool(name=f"p3_ot{_rep}", bufs=2))
            for g in range(NBLK):
                hh = p3_hh.tile([H, TBLK * BR], BF, tag="hh")
                hh4 = hh.rearrange("h (t b r) -> h t b r", b=BL, r=R)
                for b in range(BL):
                    nc.sync.dma_start(
                        out=hh4[:, :, b, :],
                        in_=hist4d[g * TBLK:(g + 1) * TBLK, :, b, 0:R].rearrange(
                            "t h r -> h t r"),
                    )
                po = p3_ps.tile([TBLK * BL, O], F32, tag="po")
                for r in range(R):
                    nc.tensor.matmul(
                        po[:],
                        hh4[:, :, :, r],
                        Wout_s[:, r * O:(r + 1) * O],
                        start=(r == 0), stop=False,
                    )
                nc.tensor.matmul(po[:], ones_s[:, 0:TBLK * BL], bout_s[:], start=False, stop=True)
                ot = p3_ot.tile([TBLK * BL, O], F32, tag="ot")
                nc.scalar.activation(out=ot[:], in_=po[:], func=Act.Copy, scale=1.0)
                nc.sync.dma_start(
                    out=out_d[g * TBLK:(g + 1) * TBLK, :, :].rearrange("t b o -> (t b) o"),
                    in_=ot[:],
                )


        for _rep in range(NREP):
            with ExitStack() as bctx:
                _emit_body(_rep, bctx)

    nc.compile()
    return nc


def _prep_inputs(x, C, W_ih, W_hh, W_rhh, bias, W_out, b_out):
    bf = ml_dtypes.bfloat16
    shared = {
        "C": np.ascontiguousarray(C).astype(bf),
        "Whh": np.ascontiguousarray(W_hh.transpose(1, 0, 2).reshape(H, R * H)).astype(bf),
        "Wrhh": np.ascontiguousarray(W_rhh.transpose(1, 0, 2).reshape(H, R * H)).astype(bf),
        "Wih": np.ascontiguousarray(W_ih.transpose(1, 0, 2).reshape(I, R * H)).astype(bf),
        "Wout": np.ascontiguousarray(
            W_out.reshape(R, H, O).transpose(1, 0, 2).reshape(H, R * O)
        ).astype(bf),
        "biasT": np.ascontiguousarray(bias.T).astype(np.float32),
        "bout": np.ascontiguousarray(b_out.reshape(1, O)).astype(bf),
    }
    in_maps = []
    for c in range(NCORES):
        xc = x[:, c * BL:(c + 1) * BL, :]                     # [T, BL, I]
        xT = np.ascontiguousarray(xc.transpose(2, 0, 1).reshape(I, TB)).astype(bf)
        m = dict(shared)
        m["xT"] = xT
        in_maps.append(m)
    return in_maps


def kernel(x, C, W_ih, W_hh, W_rhh, bias, W_out, b_out, _trace=False):
    x = np.asarray(x, np.float32)
    in_maps = _prep_inputs(
        x, np.asarray(C, np.float32), np.asarray(W_ih, np.float32),
        np.asarray(W_hh, np.float32), np.asarray(W_rhh, np.float32),
        np.asarray(bias, np.float32), np.asarray(W_out, np.float32),
        np.asarray(b_out, np.float32),
    )
    if "nc" not in _CACHE:
        _CACHE["nc"] = _build_program()
    nc = _CACHE["nc"]
    res = run_bass_kernel_spmd(nc, in_maps, list(range(NCORES)), trace=_trace)
    out = np.empty((T, B, O), np.float32)
    for c in range(NCORES):
        out[:, c * BL:(c + 1) * BL, :] = res.results[c]["out"]
    if _trace:
        return out, res
    return out



# revision 34
# speedup vs baseline: 1629.2424x; 1629.2424x over previous
"""Multi-region RNN kernel for Trainium2 (8 NeuronCores, SPMD batch-sharded).

Model (per step t):
    inp  = einsum('bi,rih->rbh', x_t, W_ih)
    loc  = einsum('rbh,rhg->rbg', H, W_hh)
    msg  = einsum('ij,ibh->jbh', C, H)
    cross= einsum('rbh,rhg->rbg', msg, W_rhh)
    H'   = tanh(inp + loc + cross + bias)
Output: stack H over t -> [T,B,R*H] @ W_out + b_out.

Distribution: pure data-parallel over batch (B=32 -> 4 per core), parameters
replicated; no cross-core communication. Per core:
  Phase 1: input drive for all t precomputed as per-region matmuls
           (W_ih[r] stationary, x^T moving), bias folded in via DVE
           tensor_scalar_add evictions into SBUF-resident double-buffered
           32-step blocks in [h,(t,b,rpad)] layout (no DRAM round trip,
           no per-step input DMA).
  Phase 2: sequential recurrence. State kept as bf16 [h=128, (b,r)=400] tiles.
           Per step: 100 loc MMs + 4 DMA-transposes (state -> region-major) +
           4 msg MMs (lhsT=H region-major, rhs=C) + 100 cross MMs, all
           accumulating in one PSUM bank slice-per-region; then DVE add of the
           staged input drive and ScalarE tanh. bf16 operands, fp32 PSUM.
  Phase 3: output projection from the bf16 state history (DRAM) with
           per-region accumulation into PSUM over (t,b)-blocks of 128;
           b_out added via a K=1 matmul of ones x b_out.
"""

import numpy as np
import ml_dtypes
from contextlib import ExitStack

import concourse.bass as bass
import concourse.bacc as bacc
import concourse.tile as tile
from concourse import mybir
from concourse.bass_utils import run_bass_kernel_spmd

T, B, I, H, R, O = 128, 32, 128, 128, 100, 64
NCORES = 8
BL = B // NCORES          # batch per core = 4
BR = BL * R               # state free size = 400, col = b*R + r
TB = T * BL               # 512
TBLK = 32                 # t-steps per phase-1/3 block -> 128 (t,b) cols
RPAD = 128                # region stride in state layout (DMA transpose needs x128)
BRP = BL * RPAD           # padded state free size = 512, col = b*RPAD + r

BF = mybir.dt.bfloat16
F32 = mybir.dt.float32
Act = mybir.ActivationFunctionType

# Enable walrus LDWEIGHTS optimization (off by default in compile_bir_kernel);
# the recurrence is weight-load bound, so LDW pipelining is the main lever.
import os as _os
if _os.environ.get("KERNEL_LDW_OPT", "0") == "1":
    import concourse.bass_utils as _bu
    if not getattr(_bu, "_ldw_opt_patched", False):
        _orig_run_command = _bu.run_command

        def _run_command_ldw(argv, **kwargs):
            argv = ["--enable-ldw-opt=true" if a == "--enable-ldw-opt=false" else a
                    for a in argv]
            return _orig_run_command(argv, **kwargs)

        _bu.run_command = _run_command_ldw
        _bu._ldw_opt_patched = True

_CACHE: dict = {}
NREP = 1   # test-only hook: repeat the whole body to measure device time deltas


def _build_program():
    nc = bacc.Bacc(None, target_bir_lowering=False)

    xT_d = nc.dram_tensor("xT", [I, TB], BF, kind="ExternalInput")        # [i,(t,b)]
    C_d = nc.dram_tensor("C", [R, R], BF, kind="ExternalInput")           # [i,j]
    Whh_d = nc.dram_tensor("Whh", [H, R * H], BF, kind="ExternalInput")   # [h,(r,g)]
    Wrhh_d = nc.dram_tensor("Wrhh", [H, R * H], BF, kind="ExternalInput")
    Wih_d = nc.dram_tensor("Wih", [I, R * H], BF, kind="ExternalInput")
    Wout_d = nc.dram_tensor("Wout", [H, R * O], BF, kind="ExternalInput")  # [h,(r,o)]
    biasT_d = nc.dram_tensor("biasT", [H, R], F32, kind="ExternalInput")
    bout_d = nc.dram_tensor("bout", [1, O], BF, kind="ExternalInput")
    out_d = nc.dram_tensor("out", [T, BL, O], F32, kind="ExternalOutput")

    with tile.TileContext(nc) as tc, ExitStack() as ctx:
        consts = ctx.enter_context(tc.tile_pool(name="consts", bufs=1))
        dram = ctx.enter_context(tc.tile_pool(name="dram", bufs=1, space="DRAM"))

        Whh_s = consts.tile([H, R * H], BF)
        nc.sync.dma_start(Whh_s[:], Whh_d[:])
        Wrhh_s = consts.tile([H, R * H], BF)
        nc.sync.dma_start(Wrhh_s[:], Wrhh_d[:])
        Wih_s = consts.tile([I, R * H], BF)
        nc.sync.dma_start(Wih_s[:], Wih_d[:])
        Wout_s = consts.tile([H, R * O], BF)
        nc.sync.dma_start(Wout_s[:], Wout_d[:])
        xT_s = consts.tile([I, TB], BF)
        nc.sync.dma_start(xT_s[:], xT_d[:])
        C_s = consts.tile([R, R], BF)
        nc.sync.dma_start(C_s[:], C_d[:])
        biasT_s = consts.tile([H, R], F32)
        nc.sync.dma_start(biasT_s[:], biasT_d[:])
        bout_s = consts.tile([1, O], BF)
        nc.sync.dma_start(bout_s[:], bout_d[:])
        ones_s = consts.tile([1, H], BF)
        nc.vector.memset(ones_s[:], 1.0)

        def _emit_body(_rep, bctx):
            hist_dram = dram.tile([T, H, BRP], BF, name=f"hist_dram{_rep}")
            hist4d = hist_dram.rearrange("t h (b r) -> t h b r", r=RPAD)

            # ---------------- Phase 1: input drive ----------------
            # Input drive stays SBUF-resident: per 32-step block, one bf16
            # tile in [h, (t, b, rpad)] layout, double-buffered so block
            # tb+1 computes while block tb's steps consume it. No DRAM
            # round trip and no per-step input DMA. PSUM->block eviction
            # runs on the mostly-idle DVE so ACT only ever runs the
            # per-step Msg copy + tanh.
            NBLK = T // TBLK
            p1_ps = bctx.enter_context(tc.tile_pool(name=f"p1_ps{_rep}", bufs=2, space="PSUM"))
            p1_st = bctx.enter_context(tc.tile_pool(name=f"p1_st{_rep}", bufs=2))
            inp_blks = []
            for tb in range(NBLK):
                blk = p1_st.tile([H, TBLK * BRP], BF, tag="p1blk")
                blk4 = blk.rearrange("h (t b r) -> h t b r", b=BL, r=RPAD)
                inp_blks.append(blk4)
                # pad lanes r in [R, RPAD) are read by the per-step DVE add
                # (through the padded state layout) but never written by the
                # region loop
                nc.vector.memset(blk4[:, :, :, R:RPAD], 0.0)
                for r in range(R):
                    ps = p1_ps.tile([H, TBLK * BL], F32, tag="p1psum")
                    nc.tensor.matmul(
                        ps[:],
                        Wih_s[:, r * H:(r + 1) * H],
                        xT_s[:, tb * TBLK * BL:(tb + 1) * TBLK * BL],
                        start=True, stop=True,
                    )
                    nc.vector.tensor_scalar_add(
                        blk4[:, :, :, r],
                        ps.rearrange("h (t b) -> h t b", b=BL),
                        biasT_s[:, r:r + 1],
                    )

            # ---------------- Phase 2: recurrence ----------------
            st_pool = bctx.enter_context(tc.tile_pool(name=f"st{_rep}", bufs=3))
            hrm_pool = bctx.enter_context(tc.tile_pool(name=f"hrm{_rep}", bufs=2))
            msg_pool = bctx.enter_context(tc.tile_pool(name=f"msgp{_rep}", bufs=2))
            pre_pool = bctx.enter_context(tc.tile_pool(name=f"prep{_rep}", bufs=2))
            ps_act = bctx.enter_context(tc.tile_pool(name=f"ps_act{_rep}", bufs=2, space="PSUM"))
            ps_msg = bctx.enter_context(tc.tile_pool(name=f"ps_msg{_rep}", bufs=2, space="PSUM"))

            Hprev = st_pool.tile([H, BRP], BF, tag="hstate")
            nc.vector.memset(Hprev[:], 0.0)

            for t in range(T):
                inp4 = inp_blks[t // TBLK]
                t_loc = t % TBLK

                pa = ps_act.tile([H, BRP], F32, tag="pa")
                paR = pa.rearrange("h (b r) -> h b r", r=RPAD)
                HprevR = Hprev.rearrange("h (b r) -> h b r", r=RPAD)

                # state -> region-major [i, (b,h)] via per-batch DMA
                # transposes, spread across both HWDGE queues
                Hrm = hrm_pool.tile([RPAD, BL * H], BF, tag="hrm")
                for b, eng in zip(range(BL),
                                  (nc.sync, nc.scalar, nc.sync, nc.scalar)):
                    eng.dma_start(
                        out=Hrm[:, b * H:(b + 1) * H],
                        in_=Hprev[:, b * RPAD:(b + 1) * RPAD],
                        transpose=True,
                    )

                # local recurrence: per-region W_hh.
                # PSUM start=True clears the whole bank's has_written flags, so
                # only the first matmul into this tile may set it; later matmuls
                # write fresh columns / accumulate based on per-element flags.
                # First half of loc runs while the transposes land; the msg
                # matmuls slot in mid-stream so their ACT eviction overlaps the
                # second loc half, and cross starts without a PE stall.
                for r in range(R // 2):
                    nc.tensor.matmul(
                        paR[:, :, r],
                        Whh_s[:, r * H:(r + 1) * H],
                        HprevR[:, :, r],
                        start=(r == 0), stop=False,
                    )

                # message: msg_b^T[h,j] = sum_i H_b[i,h] C[i,j]
                pm = ps_msg.tile([H, BRP], F32, tag="pm")
                for b in range(BL):
                    nc.tensor.matmul(
                        pm[:, b * RPAD:b * RPAD + R],
                        Hrm[0:R, b * H:(b + 1) * H],
                        C_s[:],
                        start=(b == 0), stop=(b == BL - 1),
                    )
                Msg = msg_pool.tile([H, BRP], BF, tag="msg")
                nc.scalar.activation(out=Msg[:], in_=pm[:], func=Act.Copy, scale=1.0)
                MsgR = Msg.rearrange("h (b r) -> h b r", r=RPAD)

                for r in range(R // 2, R):
                    nc.tensor.matmul(
                        paR[:, :, r],
                        Whh_s[:, r * H:(r + 1) * H],
                        HprevR[:, :, r],
                        start=False, stop=False,
                    )

                # cross term: per-region W_rhh on the mixed state
                for r in range(R):
                    nc.tensor.matmul(
                        paR[:, :, r],
                        Wrhh_s[:, r * H:(r + 1) * H],
                        MsgR[:, :, r],
                        start=False, stop=(r == R - 1),
                    )

                # add input drive + tanh, split by region halves: the first
                # half of the next step's loc matmuls only needs the first
                # half of the state, so the PE restarts while half 2 activates
                Pre = pre_pool.tile([H, BRP], F32, tag="pre")
                Hnext = st_pool.tile([H, BRP], BF, tag="hstate")
                PreR = Pre.rearrange("h (b r) -> h b r", r=RPAD)
                HnextR = Hnext.rearrange("h (b r) -> h b r", r=RPAD)
                for lo, hi in ((0, R // 2), (R // 2, RPAD)):
                    nc.vector.tensor_tensor(
                        PreR[:, :, lo:hi], paR[:, :, lo:hi],
                        inp4[:, t_loc, :, lo:hi],
                        mybir.AluOpType.add)
                    nc.scalar.activation(out=HnextR[:, :, lo:hi],
                                         in_=PreR[:, :, lo:hi], func=Act.Tanh)
                nc.gpsimd.dma_start(out=hist_dram[t, :, :], in_=Hnext[:])
                Hprev = Hnext

            # ---------------- Phase 3: output projection ----------------
            p3_hh = bctx.enter_context(tc.tile_pool(name=f"p3_hh{_rep}", bufs=1))
            p3_ps = bctx.enter_context(tc.tile_pool(name=f"p3_ps{_rep}", bufs=2, space="PSUM"))
            p3_ot = bctx.enter_context(tc.tile_pool(name=f"p3_ot{_rep}", bufs=2))
            for g in range(NBLK):
                hh = p3_hh.tile([H, TBLK * BR], BF, tag="hh")
                hh4 = hh.rearrange("h (t b r) -> h t b r", b=BL, r=R)
                for b in range(BL):
                    nc.sync.dma_start(
                        out=hh4[:, :, b, :],
                        in_=hist4d[g * TBLK:(g + 1) * TBLK, :, b, 0:R].rearrange(
                            "t h r -> h t r"),
                    )
                po = p3_ps.tile([TBLK * BL, O], F32, tag="po")
                for r in range(R):
                    nc.tensor.matmul(
                        po[:],
                        hh4[:, :, :, r],
                        Wout_s[:, r * O:(r + 1) * O],
                        start=(r == 0), stop=False,
                    )
                nc.tensor.matmul(po[:], ones_s[:, 0:TBLK * BL], bout_s[:], start=False, stop=True)
                ot = p3_ot.tile([TBLK * BL, O], F32, tag="ot")
                nc.scalar.activation(out=ot[:], in_=po[:], func=Act.Copy, scale=1.0)
                nc.sync.dma_start(
                    out=out_d[g * TBLK:(g + 1) * TBLK, :, :].rearrange("t b o -> (t b) o"),
                    in_=ot[:],
                )


        for _rep in range(NREP):
            with ExitStack() as bctx:
                _emit_body(_rep, bctx)

    nc.compile()
    return nc


def _prep_inputs(x, C, W_ih, W_hh, W_rhh, bias, W_out, b_out):
    bf = ml_dtypes.bfloat16
    shared = {
        "C": np.ascontiguousarray(C).astype(bf),
        "Whh": np.ascontiguousarray(W_hh.transpose(1, 0, 2).reshape(H, R * H)).astype(bf),
        "Wrhh": np.ascontiguousarray(W_rhh.transpose(1, 0, 2).reshape(H, R * H)).astype(bf),
        "Wih": np.ascontiguousarray(W_ih.transpose(1, 0, 2).reshape(I, R * H)).astype(bf),
        "Wout": np.ascontiguousarray(
            W_out.reshape(R, H, O).transpose(1, 0, 2).reshape(H, R * O)
        ).astype(bf),
        "biasT": np.ascontiguousarray(bias.T).astype(np.float32),
        "bout": np.ascontiguousarray(b_out.reshape(1, O)).astype(bf),
    }
    in_maps = []
    for c in range(NCORES):
        xc = x[:, c * BL:(c + 1) * BL, :]                     # [T, BL, I]
        xT = np.ascontiguousarray(xc.transpose(2, 0, 1).reshape(I, TB)).astype(bf)
        m = dict(shared)
        m["xT"] = xT
        in_maps.append(m)
    return in_maps


def kernel(x, C, W_ih, W_hh, W_rhh, bias, W_out, b_out, _trace=False):
    x = np.asarray(x, np.float32)
    in_maps = _prep_inputs(
        x, np.asarray(C, np.float32), np.asarray(W_ih, np.float32),
        np.asarray(W_hh, np.float32), np.asarray(W_rhh, np.float32),
        np.asarray(bias, np.float32), np.asarray(W_out, np.float32),
        np.asarray(b_out, np.float32),
    )
    if "nc" not in _CACHE:
        _CACHE["nc"] = _build_program()
    nc = _CACHE["nc"]
    res = run_bass_kernel_spmd(nc, in_maps, list(range(NCORES)), trace=_trace)
    out = np.empty((T, B, O), np.float32)
    for c in range(NCORES):
        out[:, c * BL:(c + 1) * BL, :] = res.results[c]["out"]
    if _trace:
        return out, res
    return out

